# revision 9
# baseline (speedup 1.0000x reference)
"""CVQNN batched policy forward on 8 Trainium2 NeuronCores — v2 (fp16).

Device layout per core: state fp16 [100 part, 6400 free], partition index
p = d(P1)*10 + d(P2) (mode pair), free = (d(F1)@640, d(F2)@64, plane@32,
b@1x32) where (P1,P2,F1,F2) is a mode-position tuple tracked by a host-side
walk planner. Gates are complex 100x100 matmuls on the partition pair
(fp16 operands, fp32 PSUM, fp16 drains on ACT/DVE). Pair changes:
'H' hop = 10 SBUF->SBUF DMAs (one per outgoing digit) split across the two
HWDGE queues; 'T' flip = 64 PE transposes. Single-mode gates and phase
diagonals are folded into neighboring pair gates on the host (same math
as the fp32 baseline).
"""
import numpy as np

B, D, CUT, LAYERS, NCORES = 256, 4, 10, 4, 8
K = D * (D - 1) // 2
C2 = CUT * CUT
BC = B // NCORES            # 32 batch per core
FT = 2 * C2 * BC            # 6400 free elems: (d1, d2, pl, b)
S_D1, S_D2, S_PL = 2 * BC * CUT, 2 * BC, BC   # strides 640, 64, 32
NMM = 12 * LAYERS + 1       # 49 pair-gate matmuls
PAIRS = [(0, 1), (0, 2), (0, 3), (1, 2), (1, 3), (2, 3)]

# ---------------------------------------------------------------- host math

def _ops():
    a = np.diag(np.sqrt(np.arange(1, CUT)), k=1).astype(np.complex128)
    return a, a.conj().T.copy(), np.arange(CUT, dtype=np.float64)


def _expm_antiherm(G):
    w, V = np.linalg.eigh(-1j * G)
    return (V * np.exp(1j * w)) @ V.conj().T


def _bs(a, adag, t, p):
    Aab = np.kron(adag, a)
    return _expm_antiherm(t * (np.exp(1j * p) * Aab - np.exp(-1j * p) * Aab.conj().T))


def build_canonical_gates(cvqnn_weights):
    """49 gates: list of (pair(a,b), M[100x100] complex) with index = da*10+db
    (a = major digit). Same folding structure as the fp32 baseline."""
    a, adag, n = _ops()
    I10 = np.eye(CUT, dtype=np.complex128)
    w = np.asarray(cvqnn_weights, np.float64)

    gates = []
    fold01 = np.eye(C2, dtype=np.complex128)
    fold2 = I10.copy()
    fold3 = I10.copy()
    for l in range(LAYERS):
        o = 0
        th1 = w[l, o:o + K]; o += K
        ph1 = w[l, o:o + K]; o += K
        vp1 = w[l, o:o + D]; o += D
        rsq = w[l, o:o + D]; o += D
        th2 = w[l, o:o + K]; o += K
        ph2 = w[l, o:o + K]; o += K
        vp2 = w[l, o:o + D]; o += D
        rd = w[l, o:o + D]; o += D
        phd = w[l, o:o + D]; o += D
        kap = w[l, o:o + D]
        U = {PAIRS[q]: _bs(a, adag, th1[q], ph1[q]) for q in range(K)}
        V = {PAIRS[q]: _bs(a, adag, th2[q], ph2[q]) for q in range(K)}
        S = [_expm_antiherm(0.5 * rsq[m] * (a @ a - adag @ adag)) for m in range(D)]
        al = rd * np.exp(1j * phd)
        Dm = [_expm_antiherm(al[m] * adag - np.conj(al[m]) * a) for m in range(D)]
        P1 = [np.diag(np.exp(1j * vp1[m] * n)) for m in range(D)]
        P2 = [np.diag(np.exp(1j * vp2[m] * n)) for m in range(D)]
        Km = [np.diag(np.exp(1j * kap[m] * n * n)) for m in range(D)]
        SQ01 = np.kron(S[0] @ P1[0], S[1] @ P1[1])
        SQ23 = np.kron(S[2] @ P1[2], S[3] @ P1[3])
        DP01 = np.kron(Dm[0] @ P2[0], Dm[1] @ P2[1])
        DP23 = np.kron(Dm[2] @ P2[2], Dm[3] @ P2[3])

        gates.append(((0, 1), U[(0, 1)] @ fold01))
        gates.append(((0, 2), U[(0, 2)] @ np.kron(I10, fold2)))
        gates.append(((0, 3), U[(0, 3)] @ np.kron(I10, fold3)))
        gates.append(((1, 2), U[(1, 2)]))
        gates.append(((1, 3), U[(1, 3)]))
        gates.append(((2, 3), SQ23 @ U[(2, 3)]))
        gates.append(((0, 1), V[(0, 1)] @ SQ01))
        gates.append(((0, 2), V[(0, 2)]))
        gates.append(((0, 3), V[(0, 3)]))
        gates.append(((1, 2), V[(1, 2)]))
        gates.append(((1, 3), V[(1, 3)]))
        gates.append(((2, 3), DP23 @ V[(2, 3)]))
        if l < LAYERS - 1:
            fold01 = np.kron(Km[0], Km[1]) @ DP01
            fold2 = Km[2]
            fold3 = Km[3]
        else:
            gates.append(((0, 1), DP01))
    assert len(gates) == NMM
    return gates


# ------------------------------------------------------------- walk planner

def plan_walk():
    """Returns (ops, finals). ops entries:
      ('M', gi, in_sw, out_sw, dr_sw)  gate gi, orientation/drain-swap flags
      ('H',)                           hop: (P1,P2,F1,F2)->(P2,F1,P1,F2)
      ('T',)                           PE flip: ->(F1,F2,P1,P2)
    finals = positions tuple after the last gate."""
    seq = []
    for l in range(LAYERS):
        seq += PAIRS + PAIRS
    seq.append((0, 1))
    pos = (0, 1, 2, 3)
    ops = []
    for idx, pair in enumerate(seq):
        assert set(pos[:2]) == set(pair), (idx, pos, pair)
        in_sw = 0 if (pos[0], pos[1]) == pair else 1
        nxt = seq[idx + 1] if idx + 1 < len(seq) else None
        if nxt is None:
            ops.append(('M', idx, in_sw, 0, 0))
            pos = (pair[0], pair[1], pos[2], pos[3])
            break
        done = False
        for out_sw in (0, 1):
            for dr_sw in (0, 1):
                o = (pair[1], pair[0]) if out_sw else pair
                f = (pos[3], pos[2]) if dr_sw else (pos[2], pos[3])
                p = (o[0], o[1], f[0], f[1])
                hop = (p[1], p[2], p[0], p[3])
                if set(hop[:2]) == set(nxt):
                    ops.append(('M', idx, in_sw, out_sw, dr_sw))
                    ops.append(('H',))
                    pos = hop
                    done = True
                    break
            if done:
                break
        if done:
            continue
        for out_sw in (0, 1):
            for dr_sw in (0, 1):
                o = (pair[1], pair[0]) if out_sw else pair
                f = (pos[3], pos[2]) if dr_sw else (pos[2], pos[3])
                p = (o[0], o[1], f[0], f[1])
                flip = (p[2], p[3], p[0], p[1])
                if set(flip[:2]) == set(nxt):
                    ops.append(('M', idx, in_sw, out_sw, dr_sw))
                    ops.append(('T',))
                    pos = flip
                    done = True
                    break
            if done:
                break
        assert done, (idx, pos, pair, nxt)
    return ops, pos


def orient(M, pair, in_sw, out_sw):
    """M canonical [out(a,b), in(a,b)] -> device matrix for given digit swaps."""
    M4 = M.reshape(CUT, CUT, CUT, CUT)
    perm = [0, 1, 2, 3]
    if out_sw:
        perm[0], perm[1] = perm[1], perm[0]
    if in_sw:
        perm[2], perm[3] = perm[3], perm[2]
    return M4.transpose(perm).reshape(C2, C2)


def gates_dram(canon, ops):
    """fp16 [100, NMM*300]: per gate UrT | (-Ui)T | UiT columns (device order)."""
    g = np.empty((C2, NMM * 3 * C2), np.float16)
    gi = 0
    for op in ops:
        if op[0] != 'M':
            continue
        _, idx, in_sw, out_sw, _ = op
        pair, M = canon[idx]
        Md = orient(M, pair, in_sw, out_sw)
        g[:, gi * 300:gi * 300 + C2] = Md.real.T.astype(np.float16)
        g[:, gi * 300 + C2:gi * 300 + 2 * C2] = (-Md.imag.T).astype(np.float16)
        g[:, gi * 300 + 2 * C2:gi * 300 + 3 * C2] = Md.imag.T.astype(np.float16)
        gi += 1
    assert gi == NMM
    return g


# ----------------------------------------------------- host state & readout

def initial_state_dev(inputs):
    """[NCORES, 100, 6400] fp16, positions (0,1,2,3):
    p = m0*10+m1, free = m2*640 + m3*64 + pl*32 + b."""
    a, adag, n = _ops()
    z = 0.5j
    S0 = _expm_antiherm(0.5 * (np.conj(z) * (a @ a) - z * (adag @ adag)))
    psi0 = S0[:, 0]
    r = np.asarray(inputs, np.float64).reshape(-1)
    wv, Vx = np.linalg.eigh(-1j * (adag - a))
    w0 = Vx.conj().T @ psi0
    psi = (np.exp(1j * np.outer(r, wv)) * w0[None, :]) @ Vx.T
    psi = psi.reshape(B, D, CUT)
    st = np.einsum('bi,bj,bk,bl->bijkl', psi[:, 0], psi[:, 1], psi[:, 2], psi[:, 3])
    # [b, m0,m1,m2,m3] -> [m0m1, m2, m3, pl, b]
    out = np.empty((NCORES, C2, FT), np.float16)
    for c in range(NCORES):
        blk = st[c * BC:(c + 1) * BC]          # [32, 10,10,10,10]
        X = blk.transpose(1, 2, 3, 4, 0)       # [m0,m1,m2,m3,b]
        X = X.reshape(C2, CUT, CUT, BC)
        Y = np.empty((C2, CUT, CUT, 2, BC), np.float32)
        Y[:, :, :, 0, :] = X.real
        Y[:, :, :, 1, :] = X.imag
        out[c] = Y.reshape(C2, FT).astype(np.float16)
    return out


def readout_weights(finals):
    """fp16 [100, 4] lhsT for readout: col m = n(digit) if mode m on the
    partition dim else 1.0."""
    P1, P2 = finals[0], finals[1]
    n = np.arange(CUT, dtype=np.float32)
    Wt = np.ones((C2, 4), np.float32)
    for p in range(C2):
        Wt[p, P1] = n[p // 10]
        Wt[p, P2] = n[p % 10]
    return Wt.astype(np.float16)


def assemble_output(routs, finals):
    """routs [NCORES][4, 3200] f32 (free = (d1,d2,b)) -> [B, 4]."""
    P1, P2, F1, F2 = finals
    n = np.arange(CUT, dtype=np.float64)
    out = np.zeros((B, D), np.float64)
    for c in range(NCORES):
        R = np.asarray(routs[c], np.float64).reshape(4, CUT, CUT, BC)
        sl = slice(c * BC, (c + 1) * BC)
        out[sl, P1] = R[P1].sum(axis=(0, 1))
        out[sl, P2] = R[P2].sum(axis=(0, 1))
        out[sl, F1] = (R[F1] * n[:, None, None]).sum(axis=(0, 1))
        out[sl, F2] = (R[F2] * n[None, :, None]).sum(axis=(0, 1))
    return out.astype(np.float32)


# ------------------------------------------------------------ numpy dev-sim

def dev_sim(state_core, canon, ops):
    """Bit-faithful-ish numpy model (fp16 quantization at drains)."""
    S = state_core.astype(np.float32).reshape(C2, CUT, CUT, 2, BC)
    gi = 0
    for op in ops:
        if op[0] == 'M':
            _, idx, in_sw, out_sw, dr_sw = op
            pair, M = canon[idx]
            Md = orient(M, pair, in_sw, out_sw)
            Mr = Md.real.astype(np.float16).astype(np.float32)
            Mi = Md.imag.astype(np.float16).astype(np.float32)
            re = S[:, :, :, 0, :].reshape(C2, -1)
            im = S[:, :, :, 1, :].reshape(C2, -1)
            nre = Mr @ re - Mi @ im
            nim = Mi @ re + Mr @ im
            N = np.empty_like(S)
            N[:, :, :, 0, :] = nre.reshape(C2, CUT, CUT, BC)
            N[:, :, :, 1, :] = nim.reshape(C2, CUT, CUT, BC)
            if dr_sw:
                N = N.transpose(0, 2, 1, 3, 4)
            S = N.astype(np.float16).astype(np.float32)
            gi += 1
        elif op[0] == 'H':
            # [vs, u, w, pl, b] -> [su, v, w, pl, b]
            X = S.reshape(CUT, CUT, CUT, CUT, 2, BC)   # [v,s,u,w,pl,b]
            S = np.ascontiguousarray(X.transpose(1, 2, 0, 3, 4, 5)).reshape(
                C2, CUT, CUT, 2, BC)
        elif op[0] == 'W':
            S = np.ascontiguousarray(S.transpose(0, 2, 1, 3, 4))
        else:  # T flip
            X = S.reshape(CUT, CUT, CUT, CUT, 2, BC)   # [p1,p2,f1,f2,pl,b]
            S = np.ascontiguousarray(X.transpose(2, 3, 0, 1, 4, 5)).reshape(
                C2, CUT, CUT, 2, BC)
    P = (S[:, :, :, 0, :] ** 2 + S[:, :, :, 1, :] ** 2).astype(np.float16)
    Wt = readout_weights((0, 0, 0, 0))  # placeholder, caller uses real finals
    return S, P


# ------------------------------------------------------------- bass program

_NC_CACHE = {}


def build_bass():
    if 0 in _NC_CACHE:
        return _NC_CACHE[0]
    import concourse.bass as bass
    import concourse.mybir as mybir
    from concourse.tile import TileContext
    F16 = mybir.dt.float16
    F32 = mybir.dt.float32

    ops, finals = plan_walk()

    nc = bass.Bass()
    d_state = nc.dram_tensor("state0", [C2, FT], F16, kind="ExternalInput")
    d_gates = nc.dram_tensor("gates", [C2, NMM * 3 * C2], F16, kind="ExternalInput")
    d_ident = nc.dram_tensor("ident", [C2, C2], F16, kind="ExternalInput")
    d_wread = nc.dram_tensor("wread", [C2, 4], F16, kind="ExternalInput")
    d_rout = nc.dram_tensor("rout", [4, C2 * BC], F32, kind="ExternalOutput")

    with TileContext(nc) as tc:
        with tc.tile_pool(name="const", bufs=1) as cpool, \
             tc.tile_pool(name="state", bufs=1) as spool, \
             tc.tile_pool(name="mm", bufs=5, space="PSUM") as mmp, \
             tc.tile_pool(name="tp", bufs=3, space="PSUM") as tpp:

            gts = cpool.tile([C2, NMM * 3 * C2], F16, tag="gates")
            ident = cpool.tile([C2, C2], F16, tag="ident")
            wread = cpool.tile([C2, 4], F16, tag="wread")
            stA = spool.tile([C2, FT], F16, tag="stA")
            stB = spool.tile([C2, FT], F16, tag="stB")
            ptile = spool.tile([C2, C2 * BC], F16, tag="probs")
            rtile = spool.tile([4, C2 * BC], F32, tag="rt")

            nc.sync.dma_start(out=stA[:, :], in_=d_state[:, :])
            nc.sync.dma_start(out=ident[:, :], in_=d_ident[:, :])
            nc.sync.dma_start(out=wread[:, :], in_=d_wread[:, :])
            nc.sync.dma_start(out=gts[:, :], in_=d_gates[:, :])

            drain_tgl = [0]

            def drain(dst_ap, src_ap):
                # 2:3 ACT:DVE split (DVE is the faster drain engine)
                if drain_tgl[0] % 5 < 2:
                    nc.scalar.copy(out=dst_ap, in_=src_ap)
                else:
                    nc.vector.tensor_copy(dst_ap, src_ap)
                drain_tgl[0] = (drain_tgl[0] + 1) % 5

            def view(t):
                return t[:, :].rearrange("p (d1 d2 pl b) -> p d1 d2 pl b",
                                         d1=CUT, d2=CUT, pl=2, b=BC)

            cur, nxt = stA, stB
            gi = 0
            hop_i = [0]
            for op in ops:
                if op[0] == 'M':
                    _, idx, in_sw, out_sw, dr_sw = op
                    Ur = gts[:, gi * 300:gi * 300 + 100]
                    nUi = gts[:, gi * 300 + 100:gi * 300 + 200]
                    Ui = gts[:, gi * 300 + 200:gi * 300 + 300]
                    gi += 1
                    cv = view(cur)
                    nv = view(nxt)
                    for k in range(CUT):          # output-block chunk
                        ps0 = mmp.tile([C2, 320], F32, tag="mm")
                        ps1 = mmp.tile([C2, 320], F32, tag="mm")
                        r_re = cv[:, k, :, 0, :]
                        r_im = cv[:, k, :, 1, :]
                        nc.tensor.matmul(ps0[:, :], Ur, r_re, start=True, stop=False)
                        nc.tensor.matmul(ps0[:, :], nUi, r_im, start=False, stop=True)
                        nc.tensor.matmul(ps1[:, :], Ui, r_re, start=True, stop=False)
                        nc.tensor.matmul(ps1[:, :], Ur, r_im, start=False, stop=True)
                        for pl, ps in ((0, ps0), (1, ps1)):
                            src = ps[:, :].rearrange("p (j b) -> p j b", j=CUT, b=BC)
                            if dr_sw:
                                drain(nv[:, :, k, pl, :], src)
                            else:
                                drain(nv[:, k, :, pl, :], src)
                elif op[0] == 'H':
                    h = hop_i[0]
                    hop_i[0] += 1
                    for v in range(CUT):
                        s_ap = cur[v * CUT:(v + 1) * CUT, :].rearrange(
                            "s (u i) -> s u i", u=CUT, i=S_D1)
                        d_ap = nxt[:, v * S_D1:(v + 1) * S_D1]
                        eng = nc.sync if (v % 10 < 6) else nc.scalar
                        eng.dma_start(out=d_ap, in_=s_ap)

                elif op[0] == 'W':
                    in_v = cur[:, :].rearrange("p (y x i) -> p x y i",
                                               y=CUT, x=CUT, i=2 * BC)
                    out_v = nxt[:, :].rearrange("p (x y i) -> p x y i",
                                                x=CUT, y=CUT, i=2 * BC)
                    nc.scalar.copy(out=out_v[:, 0:5], in_=in_v[:, 0:5])
                    nc.vector.tensor_copy(out_v[:, 5:10], in_v[:, 5:10])
                else:  # 'T' PE flip
                    cv = view(cur)
                    nv = view(nxt)
                    ftgl = [0]

                    def fdrain(dst_ap, src_ap):
                        # strict 1:1 split: the flip window is drain-bound
                        if ftgl[0] == 0:
                            nc.scalar.copy(out=dst_ap, in_=src_ap)
                        else:
                            nc.vector.tensor_copy(dst_ap, src_ap)
                        ftgl[0] ^= 1
                    for pl in range(2):
                        for bq in range(BC // 8):
                            pt = tpp.tile([C2, 800], F16, tag="tp")
                            for q in range(8):
                                bb = bq * 8 + q
                                in_ap = cur[:, :].rearrange(
                                    "p (f pl b) -> p f pl b", f=C2, pl=2, b=BC)[:, :, pl, bb]
                                nc.tensor.transpose(pt[:, q * 100:(q + 1) * 100],
                                                    in_ap, ident[:, :])
                            dst = nv[:, :, :, pl, bq * 8:bq * 8 + 8]
                            fdrain(dst, pt[:, :].rearrange(
                                "p (b p1 p2) -> p p1 p2 b", b=8, p1=CUT, p2=CUT))

                cur, nxt = nxt, cur

            # readout: P = re^2 + im^2 (fp16), then wread.T @ P
            cv = view(cur)
            pv = ptile[:, :].rearrange("p (d1 d2 b) -> p d1 d2 b",
                                       d1=CUT, d2=CUT, b=BC)
            tmp = spool.tile([C2, C2 * BC], F16, tag="probs2")
            tv = tmp[:, :].rearrange("p (d1 d2 b) -> p d1 d2 b",
                                     d1=CUT, d2=CUT, b=BC)
            nc.vector.tensor_mul(pv[:, :, :, :], cv[:, :, :, 0, :], cv[:, :, :, 0, :])
            nc.vector.tensor_mul(tv[:, :, :, :], cv[:, :, :, 1, :], cv[:, :, :, 1, :])
            nc.vector.tensor_add(ptile[:, :], ptile[:, :], tmp[:, :])
            for q in range(8):
                pr = mmp.tile([4, 400], F32, tag="mm")
                nc.tensor.matmul(pr[:, :], wread[:, :],
                                 ptile[:, q * 400:(q + 1) * 400],
                                 start=True, stop=True)
                drain(rtile[:, q * 400:(q + 1) * 400], pr[:, :])
            nc.sync.dma_start(out=d_rout[:, :], in_=rtile[:, :])

    nc.finalize()
    _legalize_waits(nc)
    _NC_CACHE[0] = nc
    return nc


def _legalize_waits(nc):
    """Walrus encodes at most one sync wait per instruction; split extras
    into preceding single-wait NoOps on the same engine."""
    import copy
    import concourse.mybir as mybir
    m = nc.m
    new_module = copy.replace(m, functions=[])
    for function in m.functions:
        new_function = copy.replace(function, blocks=[])
        new_function.set_allocations_from_list(function.allocations)
        for block in function.blocks:
            new_insts = []
            for inst in block.instructions:
                si = inst.sync_info
                if si is not None and si.on_wait and len(si.on_wait) > 1:
                    waits = list(si.on_wait)
                    for kk, w in enumerate(waits[:-1]):
                        new_insts.append(mybir.InstNoOp(
                            name=f"{inst.name}-lw{kk}",
                            engine=inst.engine,
                            sync_info=mybir.SyncInfo(on_wait=[w], on_update=[]),
                            bass_nofuse=True,
                        ))
                    inst.sync_info = mybir.SyncInfo(
                        on_wait=[waits[-1]], on_update=list(si.on_update))
                new_insts.append(inst)
            new_function.blocks.append(copy.replace(block, instructions=new_insts))
        new_module.functions.append(new_function)
    nc.m = new_module


def kernel(inputs, cvqnn_weights, batch_size):
    inputs = np.asarray(inputs)
    assert inputs.shape[0] == int(batch_size) == B
    canon = build_canonical_gates(np.asarray(cvqnn_weights))
    ops, finals = plan_walk()
    st = initial_state_dev(inputs)
    gd = gates_dram(canon, ops)
    ident = np.eye(C2, dtype=np.float16)
    wr = readout_weights(finals)

    nc = build_bass()
    from concourse.bass_utils import run_bass_kernel_spmd
    in_maps = [{"state0": st[c], "gates": gd, "ident": ident, "wread": wr}
               for c in range(NCORES)]
    res = run_bass_kernel_spmd(nc, in_maps, core_ids=list(range(NCORES)))
    routs = [res.results[c]["rout"] for c in range(NCORES)]
    return assemble_output(routs, finals)
